# revision 15
# baseline (speedup 1.0000x reference)
"""Trainium2 Bass kernel for AnnealingTopKSoftMax (top-8 masked softmax).

Computes, for each row of a [131072, 512] f32 tensor:
  out = softmax(where(mask_top8(x), x, -1e16))
which equals: exp(x)/sum(exp(top8(x))) at the top-8 positions, 0 elsewhere.

Strategy (pure data parallelism, batch axis sharded over 8 NeuronCores).
Per [128, 8, 512] block (rows on partitions, 8 row-subtiles per partition):
  v8[c] = max8(x_c)            # DVE: 8 largest per row (desc)
  e8    = exp(v8); s8 = sum(e8); r8 = 1/s8     # tiny per-row denominators
  u     = exp(x)               # ACT: two half-block ops (no bias; |x|<=~6)
  z_c   = match_replace(u_c, e8[c], 0)    # DVE: zero EXACTLY the top-8
Then (u - z) = "keep only the top-8" is computed on TWO different engines to
spread the load (fp32 matmul costs ~1.9us/subtile, gpsimd ~1.1us/subtile):
  subtiles 0..3:  psum_c = I @ u_c + (-I) @ z_c   # TensorE
                  out_c  = psum_c * r8[c]         # ACT readback + 1/s scale
  subtiles 4..7:  out_c  = u_c - z_c              # GpSimd tensor_tensor
                  out_c *= r8[c]                  # c=4 ACT copy, c=5..7 DVE
                                                  # tensor_scalar (2x mode)
match_replace replaces exactly one occurrence per needle (first match),
reproducing jax.lax.top_k's lowest-index tie-breaking exactly (the actual
input data has 4 rows with an exact f32 tie at the top-8 boundary, so exact
value matching is required; exp is injective over the top-8 value range).
Emission is ordered per engine so no queue head-of-line-blocks another:
DVE runs block n-1's match_replaces BEFORE block n's max8s, and ACT runs
block n's exps before block n-1's readbacks. Engine budget per core:
DMA ~172us (HBM roofline), DVE ~126us, PE/ACT ~122us, GpSimd ~71us.
PSUM is written only by matmul accumulation groups (start=True..stop=True);
ACT/DVE-written PSUM + matmul accumulation races on real silicon.
"""

import os
import sys
import types

import numpy as np

import concourse.bacc as bacc
import concourse.tile as tile
from concourse import mybir
from concourse.bass_utils import run_bass_kernel_spmd
from concourse.masks import make_identity


def _install_ntff_hook() -> bool:
    """Provide antenv.axon_hooks (absent in this container) so
    run_bass_kernel_spmd(trace=True) can capture NTFF profiles under axon."""
    try:
        from antenv.axon_hooks import get_axon_ntff_profile_hook  # noqa: F401

        return True
    except ImportError:
        pass
    try:
        import antenv
        from trn_agent_boot.trn_boot import _ntff_profile_via_ctypes

        hook = _ntff_profile_via_ctypes("/opt/axon/libaxon_pjrt.so")
        mod = types.ModuleType("antenv.axon_hooks")
        _h = [hook]
        mod.set_axon_ntff_profile_hook = lambda h: _h.__setitem__(0, h)
        mod.get_axon_ntff_profile_hook = lambda: _h[0]
        sys.modules["antenv.axon_hooks"] = mod
        antenv.axon_hooks = mod
        return hook is not None
    except Exception:
        return False


N_CORES = 8
BATCH = 131072
DEPTH = 512
ROWS_PER_CORE = BATCH // N_CORES  # 16384
P = 128          # SBUF partitions; rows per sub-tile
C = 8            # row-subtiles per partition per block (16KB contiguous DMA)
BLOCK_ROWS = P * C               # 1024
N_BLOCKS = ROWS_PER_CORE // BLOCK_ROWS  # 16
N_PE = 4         # subtiles 0..N_PE-1 subtract on TensorE; rest on GpSimd
N_ACT_SCALE = 2  # gpsimd-path subtiles whose 1/s scale runs on ACT (not DVE)

F32 = mybir.dt.float32
Exp = mybir.ActivationFunctionType.Exp
Copy = mybir.ActivationFunctionType.Copy


def _build(n_blocks: int = N_BLOCKS):
    rows = n_blocks * BLOCK_ROWS
    nc = bacc.Bacc(
        "TRN2", target_bir_lowering=False, debug=False, num_devices=N_CORES
    )
    x = nc.dram_tensor("x", [rows, DEPTH], F32, kind="ExternalInput")
    out = nc.dram_tensor("out", [rows, DEPTH], F32, kind="ExternalOutput")

    # row = n*1024 + p*8 + c  ->  partition p holds 8 consecutive rows per block
    xv = x.ap().rearrange("(n p c) d -> p n c d", p=P, c=C)
    ov = out.ap().rearrange("(n p c) d -> p n c d", p=P, c=C)

    with tile.TileContext(nc) as tc:
        with (
            tc.tile_pool(name="consts", bufs=1) as consts,
            tc.tile_pool(name="xs", bufs=6) as xs_pool,
            tc.tile_pool(name="us", bufs=4) as us_pool,
            tc.tile_pool(name="stats", bufs=4) as st_pool,
            tc.tile_pool(name="psum", bufs=8, space="PSUM") as ps_pool,
        ):
            ident = consts.tile([P, P], F32)
            make_identity(nc, ident[:])
            nident = consts.tile([P, P], F32)
            nc.vector.tensor_scalar_mul(nident[:], ident[:], -1.0)

            pending = None

            def phase1(n):
                """DMA in + exp + find (max8) + denominators."""
                xt = xs_pool.tile([P, C, DEPTH], F32)
                ut = us_pool.tile([P, C, DEPTH], F32)
                v8 = st_pool.tile([P, C, 8], F32)
                e8 = st_pool.tile([P, C, 8], F32)
                s8 = st_pool.tile([P, C], F32)
                r8 = st_pool.tile([P, C], F32)
                nc.sync.dma_start(out=xt[:], in_=xv[:, n, :, :])
                # exp of the raw tile in two halves (|x| <= ~6: safe in f32)
                nc.scalar.activation(
                    out=ut[:, 0 : C // 2, :].rearrange("p c d -> p (c d)"),
                    in_=xt[:, 0 : C // 2, :].rearrange("p c d -> p (c d)"),
                    func=Exp,
                )
                nc.scalar.activation(
                    out=ut[:, C // 2 : C, :].rearrange("p c d -> p (c d)"),
                    in_=xt[:, C // 2 : C, :].rearrange("p c d -> p (c d)"),
                    func=Exp,
                )
                for c in range(C):
                    nc.vector.max(out=v8[:, c, :], in_=xt[:, c, :])
                nc.scalar.activation(
                    out=e8.rearrange("p c k -> p (c k)"),
                    in_=v8.rearrange("p c k -> p (c k)"),
                    func=Exp,
                )
                nc.vector.tensor_reduce(
                    out=s8[:],
                    in_=e8[:],
                    axis=mybir.AxisListType.X,
                    op=mybir.AluOpType.add,
                )
                nc.vector.reciprocal(out=r8[:], in_=s8[:])
                return (n, xt, ut, e8, r8, [])

            def phase2_head(state):
                """The masking tail: DVE match_replaces (z_c = u_c with its
                top-8 zeroed; z reuses the raw-x buffer) chased per-subtile
                by the TensorE / GpSimd subtracts. Emitted BEFORE block n's
                max8s so PE/GpSimd unblock at the head of the DVE queue."""
                n, xt, ut, e8, r8, pts = state
                for c in range(C):
                    nc.vector.match_replace(
                        out=xt[:, c, :],
                        in_to_replace=e8[:, c, :],
                        in_values=ut[:, c, :],
                        imm_value=0.0,
                    )
                for c in range(N_PE):
                    pt = ps_pool.tile([P, DEPTH], F32)
                    pts.append(pt)
                    nc.tensor.matmul(
                        pt[:], ident[:], ut[:, c, :], start=True, stop=False
                    )
                    nc.tensor.matmul(
                        pt[:], nident[:], xt[:, c, :], start=False, stop=True
                    )
                for c in range(N_PE, C):
                    nc.gpsimd.tensor_tensor(
                        out=xt[:, c, :],
                        in0=ut[:, c, :],
                        in1=xt[:, c, :],
                        op=mybir.AluOpType.subtract,
                    )

            def phase2_rest(state):
                """The drain tail: per-row 1/s scales, PSUM readbacks and
                output DMAs. Runs two blocks behind so every dependency
                (PE matmuls, GpSimd subtracts) is long done."""
                n, xt, ut, e8, r8, pts = state
                for c in range(N_PE, N_PE + N_ACT_SCALE):
                    nc.scalar.activation(
                        out=xt[:, c, :],
                        in_=xt[:, c, :],
                        func=Copy,
                        bias=0.0,
                        scale=r8[:, c : c + 1],
                    )
                for c in range(N_PE + N_ACT_SCALE, C):
                    nc.vector.tensor_scalar_mul(
                        xt[:, c, :], xt[:, c, :], r8[:, c : c + 1]
                    )
                # PE path readback: PSUM -> SBUF fused with the 1/s scale
                for c in range(N_PE):
                    nc.scalar.activation(
                        out=xt[:, c, :],
                        in_=pts[c][:],
                        func=Copy,
                        bias=0.0,
                        scale=r8[:, c : c + 1],
                    )
                # output DMAs ride the ACT HWDGE ring (qActDynamicHW) so the
                # input stream on the SP ring never queues behind them; two
                # half-block DMAs let output stream before the block finishes
                nc.scalar.dma_start(
                    out=ov[:, n, 0 : C // 2, :], in_=xt[:, 0 : C // 2, :]
                )
                nc.scalar.dma_start(
                    out=ov[:, n, C // 2 : C, :], in_=xt[:, C // 2 : C, :]
                )

            # software-pipelined emission, split so that each engine's queue
            # is ordered by expected readiness: block n-1's match_replaces
            # lead the DVE queue (unblocking PE/GpSimd), while the
            # scale/readback/output tail runs TWO blocks behind so every
            # cross-engine dependency has at least a full block of slack
            # (8 PSUM banks = exactly two blocks of matmul groups in flight)
            prev = None   # n-1: awaiting match_replace + subtract
            prev2 = None  # n-2: awaiting scale + readback + output
            for n in range(n_blocks):
                if prev is not None:
                    phase2_head(prev)
                state = phase1(n)
                if prev2 is not None:
                    phase2_rest(prev2)
                prev2, prev = prev, state
            phase2_head(prev)
            phase2_rest(prev2)
            phase2_rest(prev)
    nc.compile()
    return nc


def kernel(**inputs: np.ndarray) -> np.ndarray:
    full = np.ascontiguousarray(inputs["inputs"], dtype=np.float32)
    assert full.shape == (BATCH, DEPTH), full.shape

    nc = _build()
    in_maps = [
        {"x": np.ascontiguousarray(full[i * ROWS_PER_CORE : (i + 1) * ROWS_PER_CORE])}
        for i in range(N_CORES)
    ]
    tr_env = os.environ.get("BASS_TRACE", "")
    trace = tr_env not in ("", "0", "false", "False")
    if trace:
        trace = _install_ntff_hook()
    try:
        res = run_bass_kernel_spmd(
            nc, in_maps, core_ids=list(range(N_CORES)), trace=trace
        )
    except Exception:
        if not trace:
            raise
        os.environ["BASS_NEVER_TRACE"] = "1"
        try:
            res = run_bass_kernel_spmd(
                nc, in_maps, core_ids=list(range(N_CORES)), trace=False
            )
        finally:
            os.environ.pop("BASS_NEVER_TRACE", None)
    kernel.last_result = res
    return np.concatenate([r["out"] for r in res.results], axis=0)


# revision 17
# speedup vs baseline: 1.1035x; 1.1035x over previous
"""Trainium2 Bass kernel for AnnealingTopKSoftMax (top-8 masked softmax).

Computes, for each row of a [131072, 512] f32 tensor:
  out = softmax(where(mask_top8(x), x, -1e16))
which equals: exp(x)/sum(exp(top8(x))) at the top-8 positions, 0 elsewhere.

Strategy (pure data parallelism, batch axis sharded over 8 NeuronCores).
Per [128, 8, 512] block (rows on partitions, 8 row-subtiles per partition):
  v8[c] = max8(x_c)            # DVE: 8 largest per row (desc)
  e8    = exp(v8); s8 = sum(e8); r8 = 1/s8     # tiny per-row denominators
  u     = exp(x)               # ACT: two half-block ops (no bias; |x|<=~6)
  z_c   = match_replace(u_c, e8[c], 0)    # DVE: zero EXACTLY the top-8
Then (u - z) = "keep only the top-8" is computed on TWO different engines to
spread the load (fp32 matmul costs ~1.9us/subtile, gpsimd ~1.1us/subtile):
  subtiles 0..3:  psum_c = I @ u_c + (-I) @ z_c   # TensorE
                  out_c  = psum_c * r8[c]         # ACT readback + 1/s scale
  subtiles 4..7:  out_c  = u_c - z_c              # GpSimd tensor_tensor
                  out_c *= r8[c]                  # c=4 ACT copy, c=5..7 DVE
                                                  # tensor_scalar (2x mode)
match_replace replaces exactly one occurrence per needle (first match),
reproducing jax.lax.top_k's lowest-index tie-breaking exactly (the actual
input data has 4 rows with an exact f32 tie at the top-8 boundary, so exact
value matching is required; exp is injective over the top-8 value range).
Emission is ordered per engine so no queue head-of-line-blocks another:
DVE runs block n-1's match_replaces BEFORE block n's max8s, and ACT runs
block n's exps before block n-1's readbacks. Engine budget per core:
DMA ~172us (HBM roofline), DVE ~126us, PE/ACT ~122us, GpSimd ~71us.
PSUM is written only by matmul accumulation groups (start=True..stop=True);
ACT/DVE-written PSUM + matmul accumulation races on real silicon.
"""

import os
import sys
import types

import numpy as np

import concourse.bacc as bacc
import concourse.tile as tile
from concourse import mybir
from concourse.bass_utils import run_bass_kernel_spmd
from concourse.masks import make_identity


def _install_ntff_hook() -> bool:
    """Provide antenv.axon_hooks (absent in this container) so
    run_bass_kernel_spmd(trace=True) can capture NTFF profiles under axon."""
    try:
        from antenv.axon_hooks import get_axon_ntff_profile_hook  # noqa: F401

        return True
    except ImportError:
        pass
    try:
        import antenv
        from trn_agent_boot.trn_boot import _ntff_profile_via_ctypes

        hook = _ntff_profile_via_ctypes("/opt/axon/libaxon_pjrt.so")
        mod = types.ModuleType("antenv.axon_hooks")
        _h = [hook]
        mod.set_axon_ntff_profile_hook = lambda h: _h.__setitem__(0, h)
        mod.get_axon_ntff_profile_hook = lambda: _h[0]
        sys.modules["antenv.axon_hooks"] = mod
        antenv.axon_hooks = mod
        return hook is not None
    except Exception:
        return False


N_CORES = 8
BATCH = 131072
DEPTH = 512
ROWS_PER_CORE = BATCH // N_CORES  # 16384
P = 128          # SBUF partitions; rows per sub-tile
C = 8            # row-subtiles per partition per block (16KB contiguous DMA)
BLOCK_ROWS = P * C               # 1024
N_BLOCKS = ROWS_PER_CORE // BLOCK_ROWS  # 16
N_PE = 3         # subtiles 0..N_PE-1 subtract on TensorE; rest on GpSimd
N_ACT_SCALE = 4  # gpsimd-path subtiles whose 1/s scale runs on ACT (not DVE)

F32 = mybir.dt.float32
Exp = mybir.ActivationFunctionType.Exp
Copy = mybir.ActivationFunctionType.Copy


def _build(n_blocks: int = N_BLOCKS):
    rows = n_blocks * BLOCK_ROWS
    nc = bacc.Bacc(
        "TRN2", target_bir_lowering=False, debug=False, num_devices=N_CORES
    )
    x = nc.dram_tensor("x", [rows, DEPTH], F32, kind="ExternalInput")
    out = nc.dram_tensor("out", [rows, DEPTH], F32, kind="ExternalOutput")

    # row = n*1024 + p*8 + c  ->  partition p holds 8 consecutive rows per block
    xv = x.ap().rearrange("(n p c) d -> p n c d", p=P, c=C)
    ov = out.ap().rearrange("(n p c) d -> p n c d", p=P, c=C)

    with tile.TileContext(nc) as tc:
        with (
            tc.tile_pool(name="consts", bufs=1) as consts,
            tc.tile_pool(name="xs", bufs=6) as xs_pool,
            tc.tile_pool(name="us", bufs=4) as us_pool,
            tc.tile_pool(name="stats", bufs=4) as st_pool,
            tc.tile_pool(name="psum", bufs=8, space="PSUM") as ps_pool,
        ):
            ident = consts.tile([P, P], F32)
            make_identity(nc, ident[:])
            nident = consts.tile([P, P], F32)
            nc.vector.tensor_scalar_mul(nident[:], ident[:], -1.0)

            pending = None

            def phase1(n):
                """DMA in + exp + find (max8) + denominators."""
                xt = xs_pool.tile([P, C, DEPTH], F32)
                ut = us_pool.tile([P, C, DEPTH], F32)
                v8 = st_pool.tile([P, C, 8], F32)
                e8 = st_pool.tile([P, C, 8], F32)
                s8 = st_pool.tile([P, C], F32)
                r8 = st_pool.tile([P, C], F32)
                nc.sync.dma_start(out=xt[:], in_=xv[:, n, :, :])
                # exp of the raw tile in two halves (|x| <= ~6: safe in f32)
                nc.scalar.activation(
                    out=ut[:, 0 : C // 2, :].rearrange("p c d -> p (c d)"),
                    in_=xt[:, 0 : C // 2, :].rearrange("p c d -> p (c d)"),
                    func=Exp,
                )
                nc.scalar.activation(
                    out=ut[:, C // 2 : C, :].rearrange("p c d -> p (c d)"),
                    in_=xt[:, C // 2 : C, :].rearrange("p c d -> p (c d)"),
                    func=Exp,
                )
                for c in range(C):
                    nc.vector.max(out=v8[:, c, :], in_=xt[:, c, :])
                nc.scalar.activation(
                    out=e8.rearrange("p c k -> p (c k)"),
                    in_=v8.rearrange("p c k -> p (c k)"),
                    func=Exp,
                )
                nc.vector.tensor_reduce(
                    out=s8[:],
                    in_=e8[:],
                    axis=mybir.AxisListType.X,
                    op=mybir.AluOpType.add,
                )
                nc.vector.reciprocal(out=r8[:], in_=s8[:])
                return (n, xt, ut, e8, r8, [])

            def phase2_head(state):
                """The masking tail: DVE match_replaces (z_c = u_c with its
                top-8 zeroed; z reuses the raw-x buffer) chased per-subtile
                by the TensorE / GpSimd subtracts. Emitted BEFORE block n's
                max8s so PE/GpSimd unblock at the head of the DVE queue."""
                n, xt, ut, e8, r8, pts = state
                for c in range(C):
                    nc.vector.match_replace(
                        out=xt[:, c, :],
                        in_to_replace=e8[:, c, :],
                        in_values=ut[:, c, :],
                        imm_value=0.0,
                    )
                for c in range(N_PE):
                    pt = ps_pool.tile([P, DEPTH], F32)
                    pts.append(pt)
                    nc.tensor.matmul(
                        pt[:], ident[:], ut[:, c, :], start=True, stop=False
                    )
                    nc.tensor.matmul(
                        pt[:], nident[:], xt[:, c, :], start=False, stop=True
                    )
                for c in range(N_PE, C):
                    nc.gpsimd.tensor_tensor(
                        out=xt[:, c, :],
                        in0=ut[:, c, :],
                        in1=xt[:, c, :],
                        op=mybir.AluOpType.subtract,
                    )

            def phase2_rest(state):
                """The drain tail: per-row 1/s scales, PSUM readbacks and
                output DMAs. Runs two blocks behind so every dependency
                (PE matmuls, GpSimd subtracts) is long done."""
                n, xt, ut, e8, r8, pts = state
                for c in range(N_PE, N_PE + N_ACT_SCALE):
                    nc.scalar.activation(
                        out=xt[:, c, :],
                        in_=xt[:, c, :],
                        func=Copy,
                        bias=0.0,
                        scale=r8[:, c : c + 1],
                    )
                for c in range(N_PE + N_ACT_SCALE, C):
                    nc.vector.tensor_scalar_mul(
                        xt[:, c, :], xt[:, c, :], r8[:, c : c + 1]
                    )
                # PE path readback: PSUM -> SBUF fused with the 1/s scale
                for c in range(N_PE):
                    nc.scalar.activation(
                        out=xt[:, c, :],
                        in_=pts[c][:],
                        func=Copy,
                        bias=0.0,
                        scale=r8[:, c : c + 1],
                    )
                # output DMAs ride the ACT HWDGE ring (qActDynamicHW) so the
                # input stream on the SP ring never queues behind them; two
                # half-block DMAs let output stream before the block finishes
                nc.scalar.dma_start(
                    out=ov[:, n, 0 : C // 2, :], in_=xt[:, 0 : C // 2, :]
                )
                nc.scalar.dma_start(
                    out=ov[:, n, C // 2 : C, :], in_=xt[:, C // 2 : C, :]
                )

            # software-pipelined emission, split so that each engine's queue
            # is ordered by expected readiness: block n-1's match_replaces
            # (chased by its PE/GpSimd subtracts) lead the DVE queue, ahead
            # of block n's max8s; block n's exps lead the ACT queue, ahead
            # of block n-1's scales/readbacks/outputs. Deeper (2-block)
            # software pipelining was measured to inflate every engine's
            # per-op time ~20% via SBUF/PSUM port contention - keep it at
            # one block.
            pending = None
            for n in range(n_blocks):
                if pending is not None:
                    phase2_head(pending)
                state = phase1(n)
                if pending is not None:
                    phase2_rest(pending)
                pending = state
            phase2_head(pending)
            phase2_rest(pending)
    nc.compile()
    return nc


def kernel(**inputs: np.ndarray) -> np.ndarray:
    full = np.ascontiguousarray(inputs["inputs"], dtype=np.float32)
    assert full.shape == (BATCH, DEPTH), full.shape

    nc = _build()
    in_maps = [
        {"x": np.ascontiguousarray(full[i * ROWS_PER_CORE : (i + 1) * ROWS_PER_CORE])}
        for i in range(N_CORES)
    ]
    tr_env = os.environ.get("BASS_TRACE", "")
    trace = tr_env not in ("", "0", "false", "False")
    if trace:
        trace = _install_ntff_hook()
    try:
        res = run_bass_kernel_spmd(
            nc, in_maps, core_ids=list(range(N_CORES)), trace=trace
        )
    except Exception:
        if not trace:
            raise
        os.environ["BASS_NEVER_TRACE"] = "1"
        try:
            res = run_bass_kernel_spmd(
                nc, in_maps, core_ids=list(range(N_CORES)), trace=False
            )
        finally:
            os.environ.pop("BASS_NEVER_TRACE", None)
    kernel.last_result = res
    return np.concatenate([r["out"] for r in res.results], axis=0)
